# revision 10
# baseline (speedup 1.0000x reference)
"""3-layer GRU (PyTorch gate order) + BatchNorm1d (batch stats) + FC + sigmoid.

Strategy: data-parallel over batch across 8 NeuronCores (64 rows/core),
GRU weights replicated.  Per core and per layer:

  Phase A: gx = in_seq @ WihT (+ bias) for all T*64 tokens, written to DRAM.
           lhsT = transposed input sequence (128-token blocks), rhs = WihT
           chunks resident in SBUF, fp32r matmuls at full PE rate (N=512).
  Phase R: sequential recurrence.  gh = h @ WhhT computed with lhsT = hT
           (hidden-major state, 8 chunks of [128, 64]), rhs = WhhT chunks,
           PSUM [64, 3072].  bhh_n is accumulated into the n-gate PSUM via a
           K=1 ones-matmul.  Elementwise on DVE/ACT in [64, *] layout, then
           8 PE transposes regenerate hT for the next step.

BN batch stats: per-core partial sum/sumsq via ones-matmuls -> AllReduce
across the 8 cores -> BN+FC folded into y = h63 @ (gamma*rstd*fcW) + C.
"""

import numpy as np

import concourse.bacc as bacc
import concourse.bass as bass
import concourse.mybir as mybir
import concourse.tile as tile
from concourse.bass_utils import run_bass_kernel_spmd
N_CORES = 8
B, T, F, H = 512, 64, 64, 1024
BL = B // N_CORES          # 64 batch rows per core
G = 3 * H                  # 3072 gates
KH = H // 128              # 8 contraction chunks
NCH = G // 512             # 6 output chunks of 512
EPS = 1e-5

F32 = mybir.dt.float32
F32R = mybir.dt.float32r
AOP = mybir.AluOpType
ACTF = mybir.ActivationFunctionType


def _emit(nc, tc, seq_len):
    nch_blocks = seq_len * BL // 128  # 128-token blocks per layer in phase A

    xT = nc.dram_tensor("xT", [F, seq_len, BL], F32R, kind="ExternalInput").ap()
    wih = [
        nc.dram_tensor("wih0T", [F, G], F32R, kind="ExternalInput").ap(),
        nc.dram_tensor("wih1T", [H, G], F32R, kind="ExternalInput").ap(),
        nc.dram_tensor("wih2T", [H, G], F32R, kind="ExternalInput").ap(),
    ]
    whh = [
        nc.dram_tensor(f"whh{i}T", [H, G], F32R, kind="ExternalInput").ap()
        for i in range(3)
    ]
    # bias_bc[l]: [128, 3072] broadcast of (bih + [bhh_rz, 0]) -- added to gx.
    bias_bc = [
        nc.dram_tensor(f"bias{i}", [128, G], F32, kind="ExternalInput").ap()
        for i in range(3)
    ]
    # misc[l]: [1, 1088] = bhh_n (1024) ++ ones (64)
    misc = [
        nc.dram_tensor(f"misc{i}", [1, 1088], F32R, kind="ExternalInput").ap()
        for i in range(3)
    ]
    gamma_pm = nc.dram_tensor("gamma_pm", [128, KH], F32, kind="ExternalInput").ap()
    beta_pm = nc.dram_tensor("beta_pm", [128, KH], F32, kind="ExternalInput").ap()
    fcw_pm = nc.dram_tensor("fcw_pm", [128, KH], F32, kind="ExternalInput").ap()
    fcb_d = nc.dram_tensor("fcb", [1, 1], F32, kind="ExternalInput").ap()
    # const_d: [eye(128) | ones(128x64) | zeros(128x64)]
    const_d = nc.dram_tensor("const_d", [128, 256], F32R,
                             kind="ExternalInput").ap()

    gx_d = nc.dram_tensor("gx_d", [seq_len * BL, G], F32).ap()
    hseqT_d = nc.dram_tensor("hseqT_d", [KH, 128, seq_len, BL], F32R).ap()
    bn_in = nc.dram_tensor("bn_in", [2 * H], F32).ap()
    bn_out = nc.dram_tensor("bn_out", [2 * H], F32, addr_space="Shared").ap()
    out_d = nc.dram_tensor("out", [BL], F32, kind="ExternalOutput").ap()

    const_pool = tc.alloc_tile_pool(name="const", bufs=1)
    constt = const_pool.tile([128, 256], F32R, name="constt")
    nc.sync.dma_start(constt[:], const_d[:])
    ident = constt[:, 0:128]

    # weight chunk pool: 8 chunks of [128, 3072] rotate through phases
    wpool = tc.alloc_tile_pool(name="wpool", bufs=KH)
    hpool = tc.alloc_tile_pool(name="hpool", bufs=2)
    htbpool = tc.alloc_tile_pool(name="htb", bufs=2 * KH + 2)
    biaspool = tc.alloc_tile_pool(name="biasp", bufs=1)
    miscpool = tc.alloc_tile_pool(name="miscp", bufs=1)

    def load_w_chunks(src):
        tiles = []
        for k in range(KH):
            wt = wpool.tile([128, G], F32R, name=f"w_{k}", tag="w")
            nc.sync.dma_start(wt[:], src[k * 128:(k + 1) * 128, :])
            tiles.append((wt, 128))
        return tiles

    def phase_r(layer):
        """Sequential GRU recurrence for one layer."""
        wtiles = load_w_chunks(whh[layer])
        misc_t = miscpool.tile([1, 1088], F32R, name=f"misc_l{layer}", tag="misc")
        nc.sync.dma_start(misc_t[:], misc[layer][:])
        ones_ap = misc_t[0:1, 1024:1088]

        h_prev = hpool.tile([BL, H], F32, name=f"h_init_l{layer}", tag="h")
        nc.gpsimd.memset(h_prev[:], 0.0)
        ht_prev = [constt[:, 192:256] for _ in range(KH)]

        with (
            tc.tile_pool(name="gxp", bufs=2) as gx_pool,
            tc.tile_pool(name="sp", bufs=1) as s_pool,
            tc.tile_pool(name="zhp", bufs=2) as zh_pool,
            tc.tile_pool(name="tmpp", bufs=3) as tmp_pool,
            tc.tile_pool(name="ghp", bufs=1, space="PSUM") as gh_pool,
            tc.tile_pool(name="trp", bufs=2, space="PSUM") as tr_pool,
        ):
            for t in range(seq_len):
                gxt = gx_pool.tile([BL, G], F32, name=f"gx_{t}", tag="gx")
                nc.sync.dma_start(gxt[:], gx_d[t * BL:(t + 1) * BL, :])

                gh = gh_pool.tile([BL, G], F32, name=f"gh_{t}", tag="gh")
                for n in range(NCH):
                    nsl = slice(n * 512, (n + 1) * 512)
                    for k in range(KH):
                        wt, _ = wtiles[k]
                        nc.tensor.matmul(
                            gh[:, nsl], ht_prev[k][:],
                            wt[:, nsl],
                            start=(k == 0), stop=(k == KH - 1 and n < 4))
                    if n >= 4:
                        # accumulate bhh_n via K=1 ones-matmul
                        bsl = slice((n - 4) * 512, (n - 3) * 512)
                        nc.tensor.matmul(
                            gh[:, nsl], ones_ap[:, :BL],
                            misc_t[0:1, bsl],
                            start=False, stop=True)

                # s = sigmoid(gx_rz + gh_rz)   (gx carries bih+bhh for r,z)
                s = s_pool.tile([BL, 2 * H], F32, name=f"s_{t}", tag="s")
                nc.vector.tensor_tensor(s[:], gxt[:, 0:2 * H], gh[:, 0:2 * H],
                                        AOP.add)
                nc.scalar.activation(s[:], s[:], ACTF.Sigmoid)
                # zh = z * h_prev ; omz = 1 - z (in place over z)
                zh = zh_pool.tile([BL, H], F32, name=f"zh_{t}", tag="zh")
                nc.vector.tensor_tensor(zh[:], s[:, H:2 * H], h_prev[:], AOP.mult)
                nc.scalar.activation(s[:, H:2 * H], s[:, H:2 * H], ACTF.Identity,
                                     bias=1.0, scale=-1.0)
                # n = tanh(gx_n + r * (gh_n + bhh_n))  (bhh_n already in PSUM)
                t1 = tmp_pool.tile([BL, H], F32, name=f"t1_{t}", tag="tmp")
                nc.vector.tensor_tensor(t1[:], s[:, 0:H], gh[:, 2 * H:3 * H],
                                        AOP.mult)
                t2 = tmp_pool.tile([BL, H], F32, name=f"t2_{t}", tag="tmp")
                nc.vector.tensor_tensor(t2[:], gxt[:, 2 * H:3 * H], t1[:], AOP.add)
                nc.scalar.activation(s[:, 0:H], t2[:], ACTF.Tanh)
                # h_new = zh + (1-z)*n
                t3 = tmp_pool.tile([BL, H], F32, name=f"t3_{t}", tag="tmp")
                nc.vector.tensor_tensor(t3[:], s[:, H:2 * H], s[:, 0:H], AOP.mult)
                h_new = hpool.tile([BL, H], F32R, name=f"h_{t}", tag="h")
                nc.vector.tensor_tensor(h_new[:], zh[:], t3[:], AOP.add)

                ht_new = []
                for k in range(KH):
                    tp = tr_pool.tile([128, BL], F32R, name=f"tr_{t}_{k}", tag="tr")
                    nc.tensor.transpose(tp[:], h_new[:, k * 128:(k + 1) * 128],
                                        ident[0:BL, 0:BL])
                    ht = htbpool.tile([128, BL], F32R, name=f"ht_{t}_{k}",
                                      tag="htb")
                    nc.scalar.copy(ht[:], tp[:])
                    if layer < 2 or t == seq_len - 1:
                        nc.sync.dma_start(hseqT_d[k, :, t, :], ht[:])
                    ht_new.append(ht)
                ht_prev = ht_new
                h_prev = h_new
        return h_prev

    # ---- emit the three layers ----
    # NB: for layer 0 phase A the "weight" source is wih0T with K=F=64.
    def phase_a0():
        wt = wpool.tile([128, G], F32R, name="w_l0", tag="w")
        nc.sync.dma_start(wt[:F, :], wih[0][:])
        bias_t = biaspool.tile([128, G], F32, name="bias_l0", tag="bias")
        nc.sync.dma_start(bias_t[:], bias_bc[0][:])
        with (
            tc.tile_pool(name="alhs0", bufs=4) as alhs_pool,
            tc.tile_pool(name="apsum0", bufs=2, space="PSUM") as apsum_pool,
            tc.tile_pool(name="gstage0", bufs=3) as gstage_pool,
        ):
            for j in range(nch_blocks):
                lt = alhs_pool.tile([128, 128], F32R, name=f"a0lhs_{j}",
                                    tag="alhs")
                nc.sync.dma_start(
                    lt[:F, :],
                    xT[:, 2 * j:2 * j + 2, :].rearrange("f t b -> f (t b)"))
                for n in range(NCH):
                    nsl = slice(n * 512, (n + 1) * 512)
                    ps = apsum_pool.tile([128, 512], F32, name=f"a0ps_{j}_{n}",
                                         tag="apsum")
                    nc.tensor.matmul(ps[:], lt[:F, :],
                                     wt[:F, nsl],
                                     start=True, stop=True)
                    st = gstage_pool.tile([128, 512], F32, name=f"a0st_{j}_{n}",
                                          tag="gst")
                    nc.vector.tensor_tensor(st[:], ps[:], bias_t[:, nsl], AOP.add)
                    nc.sync.dma_start(gx_d[j * 128:(j + 1) * 128, nsl], st[:])

    def phase_a_l(layer):
        wtiles = load_w_chunks(wih[layer])
        bias_t = biaspool.tile([128, G], F32, name=f"bias_l{layer}", tag="bias")
        nc.sync.dma_start(bias_t[:], bias_bc[layer][:])
        with (
            tc.tile_pool(name=f"alhs{layer}", bufs=2 * KH) as alhs_pool,
            tc.tile_pool(name=f"apsum{layer}", bufs=2, space="PSUM") as apsum_pool,
            tc.tile_pool(name=f"gstage{layer}", bufs=3) as gstage_pool,
        ):
            for j in range(nch_blocks):
                lhs = []
                for k in range(KH):
                    lt = alhs_pool.tile([128, 128], F32R,
                                        name=f"alhs{layer}_{j}_{k}", tag="alhs")
                    nc.sync.dma_start(
                        lt[:],
                        hseqT_d[k, :, 2 * j:2 * j + 2, :].rearrange(
                            "p t b -> p (t b)"))
                    lhs.append(lt)
                for n in range(NCH):
                    nsl = slice(n * 512, (n + 1) * 512)
                    ps = apsum_pool.tile([128, 512], F32,
                                         name=f"aps{layer}_{j}_{n}", tag="apsum")
                    for k in range(KH):
                        wt, _ = wtiles[k]
                        nc.tensor.matmul(ps[:], lhs[k][:],
                                         wt[:, nsl],
                                         start=(k == 0), stop=(k == KH - 1))
                    st = gstage_pool.tile([128, 512], F32,
                                          name=f"ast{layer}_{j}_{n}", tag="gst")
                    nc.vector.tensor_tensor(st[:], ps[:], bias_t[:, nsl], AOP.add)
                    nc.sync.dma_start(gx_d[j * 128:(j + 1) * 128, nsl], st[:])

    phase_a0()
    phase_r(0)
    phase_a_l(1)
    phase_r(1)
    phase_a_l(2)
    h_last = phase_r(2)

    # ---- BatchNorm stats + BN/FC folded head ----
    with (
        tc.tile_pool(name="bnps", bufs=1, space="PSUM") as bn_psum,
        tc.tile_pool(name="bnsb", bufs=1) as bn_sb,
    ):
        ones_col = constt[0:BL, 128:129].bitcast(F32)
        h_sq = bn_sb.tile([BL, H], F32, name="h_sq")
        nc.scalar.activation(h_sq[:], h_last[:], ACTF.Square)

        stats_ps = bn_psum.tile([128, 2 * KH], F32, name="stats_ps", tag="bnp")
        for k in range(KH):
            ksl = slice(k * 128, (k + 1) * 128)
            nc.tensor.matmul(stats_ps[:, k:k + 1],
                             h_last[:, ksl].bitcast(F32),
                             ones_col, start=True, stop=True)
            nc.tensor.matmul(stats_ps[:, KH + k:KH + k + 1],
                             h_sq[:, ksl],
                             ones_col, start=True, stop=True)
        stats_sb = bn_sb.tile([128, 2 * KH], F32, name="stats_sb")
        nc.scalar.copy(stats_sb[:], stats_ps[:])
        nc.sync.dma_start(bn_in.rearrange("(p f) -> p f", p=128), stats_sb[:])
        nc.gpsimd.collective_compute(
            "AllReduce", AOP.add,
            replica_groups=[list(range(N_CORES))],
            ins=[bn_in[:]], outs=[bn_out[:]])
        agg = bn_sb.tile([128, 2 * KH], F32, name="agg")
        nc.sync.dma_start(agg[:], bn_out.rearrange("(p f) -> p f", p=128))

        gpm = bn_sb.tile([128, KH], F32, name="gpm")
        nc.sync.dma_start(gpm[:], gamma_pm[:])
        bpm = bn_sb.tile([128, KH], F32, name="bpm")
        nc.sync.dma_start(bpm[:], beta_pm[:])
        wpm = bn_sb.tile([128, KH], F32, name="wpm")
        nc.sync.dma_start(wpm[:], fcw_pm[:])
        fcb_t = bn_sb.tile([1, 1], F32, name="fcb_t")
        nc.sync.dma_start(fcb_t[:], fcb_d[:])

        mu = bn_sb.tile([128, KH], F32, name="mu")
        nc.scalar.mul(mu[:], agg[:, 0:KH], 1.0 / B)
        ex2 = bn_sb.tile([128, KH], F32, name="ex2")
        nc.scalar.mul(ex2[:], agg[:, KH:2 * KH], 1.0 / B)
        musq = bn_sb.tile([128, KH], F32, name="musq")
        nc.vector.tensor_tensor(musq[:], mu[:], mu[:], AOP.mult)
        var = bn_sb.tile([128, KH], F32, name="var")
        nc.vector.tensor_tensor(var[:], ex2[:], musq[:], AOP.subtract)
        eps_t = bn_sb.tile([128, 1], F32, name="eps_t")
        nc.gpsimd.memset(eps_t[:], EPS)
        std = bn_sb.tile([128, KH], F32, name="std")
        nc.scalar.activation(std[:], var[:], ACTF.Sqrt, bias=eps_t[:])
        rstd = bn_sb.tile([128, KH], F32, name="rstd")
        nc.vector.reciprocal(rstd[:], std[:])
        scoef = bn_sb.tile([128, KH], F32, name="scoef")
        nc.vector.tensor_tensor(scoef[:], rstd[:], gpm[:], AOP.mult)
        sw = bn_sb.tile([128, KH], F32, name="sw")
        nc.vector.tensor_tensor(sw[:], scoef[:], wpm[:], AOP.mult)
        ms = bn_sb.tile([128, KH], F32, name="ms")
        nc.vector.tensor_tensor(ms[:], mu[:], scoef[:], AOP.mult)
        d = bn_sb.tile([128, KH], F32, name="d")
        nc.vector.tensor_tensor(d[:], bpm[:], ms[:], AOP.subtract)
        dw = bn_sb.tile([128, KH], F32, name="dw")
        nc.vector.tensor_tensor(dw[:], d[:], wpm[:], AOP.mult)
        dw1 = bn_sb.tile([128, 1], F32, name="dw1")
        nc.vector.reduce_sum(dw1[:], dw[:], mybir.AxisListType.X)
        ones128 = constt[:, 128:129].bitcast(F32)
        c_ps = bn_psum.tile([1, 1], F32, name="c_ps", tag="bnc")
        nc.tensor.matmul(c_ps[:], dw1[:],
                         ones128, start=True, stop=True)
        c_sb = bn_sb.tile([1, 1], F32, name="c_sb")
        nc.vector.tensor_tensor(c_sb[:], c_ps[:], fcb_t[:], AOP.add)

        # y = h63 @ sw + C   via hT63 chunks (stored to hseqT_d at t=T-1)
        y_ps = bn_psum.tile([BL, 1], F32, name="y_ps", tag="bny")
        ht63 = []
        for k in range(KH):
            htk = bn_sb.tile([128, BL], F32, name=f"ht63_{k}")
            nc.sync.dma_start(htk[:], hseqT_d[k, :, seq_len - 1, :].bitcast(F32))
            ht63.append(htk)
        for k in range(KH):
            nc.tensor.matmul(y_ps[:], ht63[k][:],
                             sw[:, k:k + 1],
                             start=(k == 0), stop=False)
        onesb = constt[0:1, 128:128 + BL].bitcast(F32)
        nc.tensor.matmul(y_ps[:], onesb,
                         c_sb[:], start=False, stop=True)
        res = bn_sb.tile([BL, 1], F32, name="res")
        nc.scalar.activation(res[:], y_ps[:], ACTF.Sigmoid)
        nc.sync.dma_start(out_d.rearrange("(p f) -> p f", f=1), res[:])

    miscpool.release()
    biaspool.release()
    htbpool.release()
    hpool.release()
    wpool.release()
    const_pool.release()


_PROGRAM_CACHE = {}


def build_program(seq_len=T):
    if seq_len in _PROGRAM_CACHE:
        return _PROGRAM_CACHE[seq_len]
    nc = bacc.Bacc("TRN2", target_bir_lowering=False, debug=False,
                   num_devices=N_CORES)
    with nc.allow_low_precision(reason="fp32r state/operands are intentional"):
        with tile.TileContext(nc) as tc:
            _emit(nc, tc, seq_len)
    nc.compile()
    _PROGRAM_CACHE[seq_len] = nc
    return nc


def make_in_maps(inputs, seq_len=T):
    f32 = np.float32

    def prep_shared():
        m = {}
        m["wih0T"] = np.ascontiguousarray(inputs["Wih0"].T, dtype=f32)
        m["wih1T"] = np.ascontiguousarray(inputs["Wih1"].T, dtype=f32)
        m["wih2T"] = np.ascontiguousarray(inputs["Wih2"].T, dtype=f32)
        for i in range(3):
            m[f"whh{i}T"] = np.ascontiguousarray(
                inputs[f"Whh{i}"].T, dtype=f32)
            bih = np.asarray(inputs[f"bih{i}"], dtype=f32)
            bhh = np.asarray(inputs[f"bhh{i}"], dtype=f32)
            bias = bih.copy()
            bias[:2 * H] += bhh[:2 * H]
            m[f"bias{i}"] = np.ascontiguousarray(
                np.broadcast_to(bias, (128, G)), dtype=f32)
            misc = np.zeros((1, 1088), dtype=f32)
            misc[0, :H] = bhh[2 * H:]
            misc[0, H:H + 64] = 1.0
            m[f"misc{i}"] = misc
        for name, key in (("gamma_pm", "gamma"), ("beta_pm", "beta")):
            v = np.asarray(inputs[key], dtype=f32)
            m[name] = np.ascontiguousarray(v.reshape(KH, 128).T)
        fcw = np.asarray(inputs["fcW"], dtype=f32).reshape(H)
        m["fcw_pm"] = np.ascontiguousarray(fcw.reshape(KH, 128).T)
        m["fcb"] = np.asarray(inputs["fcb"], dtype=f32).reshape(1, 1)
        cd = np.zeros((128, 256), dtype=f32)
        cd[:, :128] = np.eye(128, dtype=f32)
        cd[:, 128:192] = 1.0
        m["const_d"] = cd
        return m

    shared = prep_shared()
    x = np.asarray(inputs["x"], dtype=f32)
    in_maps = []
    for c in range(N_CORES):
        xs = x[c * BL:(c + 1) * BL, :seq_len, :]          # [BL, T, F]
        xT_c = np.ascontiguousarray(xs.transpose(2, 1, 0))  # [F, T, BL]
        m = dict(shared)
        m["xT"] = xT_c
        in_maps.append(m)
    return in_maps


def kernel(**inputs):
    nc = build_program(T)
    in_maps = make_in_maps(inputs, T)
    res = run_bass_kernel_spmd(nc, in_maps, list(range(N_CORES)))
    out = np.concatenate([res.results[c]["out"] for c in range(N_CORES)])
    return out.astype(np.float32)


# revision 11
# speedup vs baseline: 219.1924x; 219.1924x over previous
"""3-layer GRU (PyTorch gate order) + BatchNorm1d (batch stats) + FC + sigmoid.

Strategy: data-parallel over batch across 8 NeuronCores (64 rows/core),
GRU weights replicated.  Per core and per layer:

  Phase A: gx = in_seq @ WihT (+ bias) for all T*64 tokens, written to DRAM.
           lhsT = transposed input sequence (128-token blocks), rhs = WihT
           chunks resident in SBUF, fp32r matmuls at full PE rate (N=512).
  Phase R: sequential recurrence.  gh = h @ WhhT computed with lhsT = hT
           (hidden-major state, 8 chunks of [128, 64]), rhs = WhhT chunks,
           PSUM [64, 3072].  bhh_n is accumulated into the n-gate PSUM via a
           K=1 ones-matmul.  Elementwise on DVE/ACT in [64, *] layout, then
           8 PE transposes regenerate hT for the next step.

BN batch stats: per-core partial sum/sumsq via ones-matmuls -> AllReduce
across the 8 cores -> BN+FC folded into y = h63 @ (gamma*rstd*fcW) + C.
"""

import numpy as np

import concourse.bacc as bacc
import concourse.bass as bass
import concourse.mybir as mybir
import concourse.tile as tile
from concourse.bass_utils import run_bass_kernel_spmd
N_CORES = 8
B, T, F, H = 512, 64, 64, 1024
BL = B // N_CORES          # 64 batch rows per core
G = 3 * H                  # 3072 gates
KH = H // 128              # 8 contraction chunks
NCH = G // 512             # 6 output chunks of 512
EPS = 1e-5

F32 = mybir.dt.float32
F32R = mybir.dt.float32r
AOP = mybir.AluOpType
ACTF = mybir.ActivationFunctionType


def _emit(nc, tc, seq_len):
    nch_blocks = seq_len * BL // 128  # 128-token blocks per layer in phase A

    xT = nc.dram_tensor("xT", [F, seq_len, BL], F32R, kind="ExternalInput").ap()
    wih = [
        nc.dram_tensor("wih0T", [F, G], F32R, kind="ExternalInput").ap(),
        nc.dram_tensor("wih1T", [H, G], F32R, kind="ExternalInput").ap(),
        nc.dram_tensor("wih2T", [H, G], F32R, kind="ExternalInput").ap(),
    ]
    whh = [
        nc.dram_tensor(f"whh{i}T", [H, G], F32R, kind="ExternalInput").ap()
        for i in range(3)
    ]
    # bias_bc[l]: [128, 3072] broadcast of (bih + [bhh_rz, 0]) -- added to gx.
    bias_bc = [
        nc.dram_tensor(f"bias{i}", [128, G], F32, kind="ExternalInput").ap()
        for i in range(3)
    ]
    # misc[l]: [1, 1088] = bhh_n (1024) ++ ones (64)
    misc = [
        nc.dram_tensor(f"misc{i}", [1, 1088], F32R, kind="ExternalInput").ap()
        for i in range(3)
    ]
    gamma_pm = nc.dram_tensor("gamma_pm", [128, KH], F32, kind="ExternalInput").ap()
    beta_pm = nc.dram_tensor("beta_pm", [128, KH], F32, kind="ExternalInput").ap()
    fcw_pm = nc.dram_tensor("fcw_pm", [128, KH], F32, kind="ExternalInput").ap()
    fcb_d = nc.dram_tensor("fcb", [1, 1], F32, kind="ExternalInput").ap()
    # const_d: [eye(128) | ones(128x64) | zeros(128x64)]
    const_d = nc.dram_tensor("const_d", [128, 256], F32R,
                             kind="ExternalInput").ap()

    gx_d = nc.dram_tensor("gx_d", [seq_len * BL, G], F32).ap()
    hseqT_d = nc.dram_tensor("hseqT_d", [KH, 128, seq_len, BL], F32R).ap()
    bn_in = nc.dram_tensor("bn_in", [2 * H], F32).ap()
    bn_out = nc.dram_tensor("bn_out", [2 * H], F32, addr_space="Shared").ap()
    out_d = nc.dram_tensor("out", [BL], F32, kind="ExternalOutput").ap()

    const_pool = tc.alloc_tile_pool(name="const", bufs=1)
    constt = const_pool.tile([128, 256], F32R, name="constt")
    nc.sync.dma_start(constt[:], const_d[:])
    ident = constt[:, 0:128]

    # weight chunk pool: 8 chunks of [128, 3072] rotate through phases
    wpool = tc.alloc_tile_pool(name="wpool", bufs=KH)
    hpool = tc.alloc_tile_pool(name="hpool", bufs=2)
    htbpool = tc.alloc_tile_pool(name="htb", bufs=2 * KH + 2)
    biaspool = tc.alloc_tile_pool(name="biasp", bufs=1)
    miscpool = tc.alloc_tile_pool(name="miscp", bufs=1)

    def load_w_chunks(src):
        tiles = []
        for k in range(KH):
            wt = wpool.tile([128, G], F32R, name=f"w_{k}", tag="w")
            nc.sync.dma_start(wt[:], src[k * 128:(k + 1) * 128, :])
            tiles.append((wt, 128))
        return tiles

    def phase_r(layer):
        """Sequential GRU recurrence for one layer."""
        wtiles = load_w_chunks(whh[layer])
        misc_t = miscpool.tile([1, 1088], F32R, name=f"misc_l{layer}", tag="misc")
        nc.sync.dma_start(misc_t[:], misc[layer][:])
        ones_ap = misc_t[0:1, 1024:1088]

        h_prev = hpool.tile([BL, H], F32, name=f"h_init_l{layer}", tag="h")
        nc.gpsimd.memset(h_prev[:], 0.0)
        ht_prev = [constt[:, 192:256] for _ in range(KH)]

        with (
            tc.tile_pool(name="gxp", bufs=2) as gx_pool,
            tc.tile_pool(name="sp", bufs=1) as s_pool,
            tc.tile_pool(name="zhp", bufs=2) as zh_pool,
            tc.tile_pool(name="tmpp", bufs=4) as tmp_pool,
            tc.tile_pool(name="ghp", bufs=1, space="PSUM") as gh_pool,
            tc.tile_pool(name="trp", bufs=2, space="PSUM") as tr_pool,
        ):
            for t in range(seq_len):
                gxt = gx_pool.tile([BL, G], F32, name=f"gx_{t}", tag="gx")
                nc.sync.dma_start(gxt[:], gx_d[t * BL:(t + 1) * BL, :])

                gh = gh_pool.tile([BL, G], F32, name=f"gh_{t}", tag="gh")
                for n in range(NCH):
                    nsl = slice(n * 512, (n + 1) * 512)
                    for k in range(KH):
                        wt, _ = wtiles[k]
                        nc.tensor.matmul(
                            gh[:, nsl], ht_prev[k][:],
                            wt[:, nsl],
                            start=(k == 0), stop=(k == KH - 1 and n < 4))
                    if n >= 4:
                        # accumulate bhh_n via K=1 ones-matmul
                        bsl = slice((n - 4) * 512, (n - 3) * 512)
                        nc.tensor.matmul(
                            gh[:, nsl], ones_ap[:, :BL],
                            misc_t[0:1, bsl],
                            start=False, stop=True)

                # r = sigmoid(gx_r + gh_r) first (critical path to tanh),
                # then z; zh/omz overlap the n-gate chain.
                s = s_pool.tile([BL, 2 * H], F32, name=f"s_{t}", tag="s")
                nc.vector.tensor_tensor(s[:, 0:H], gxt[:, 0:H], gh[:, 0:H],
                                        AOP.add)
                nc.scalar.activation(s[:, 0:H], s[:, 0:H], ACTF.Sigmoid)
                nc.vector.tensor_tensor(s[:, H:2 * H], gxt[:, H:2 * H],
                                        gh[:, H:2 * H], AOP.add)
                nc.scalar.activation(s[:, H:2 * H], s[:, H:2 * H], ACTF.Sigmoid)
                zh = zh_pool.tile([BL, H], F32, name=f"zh_{t}", tag="zh")
                nc.vector.tensor_tensor(zh[:], s[:, H:2 * H], h_prev[:], AOP.mult)
                nc.scalar.activation(s[:, H:2 * H], s[:, H:2 * H], ACTF.Identity,
                                     bias=1.0, scale=-1.0)
                # n-gate + h update in two hidden halves; PE transposes of a
                # finished half overlap DVE/ACT work on the other half.
                h_new = hpool.tile([BL, H], F32R, name=f"h_{t}", tag="h")
                ht_new = [None] * KH
                HH = H // 2
                for hf in range(2):
                    hs = slice(hf * HH, (hf + 1) * HH)
                    gn = slice(2 * H + hf * HH, 2 * H + (hf + 1) * HH)
                    t1 = tmp_pool.tile([BL, HH], F32, name=f"t1_{t}_{hf}",
                                       tag="tmp")
                    nc.vector.tensor_tensor(t1[:], s[:, hs], gh[:, gn], AOP.mult)
                    t2 = tmp_pool.tile([BL, HH], F32, name=f"t2_{t}_{hf}",
                                       tag="tmp")
                    nc.vector.tensor_tensor(t2[:], gxt[:, gn], t1[:], AOP.add)
                    nc.scalar.activation(s[:, hs], t2[:], ACTF.Tanh)
                    t3 = tmp_pool.tile([BL, HH], F32, name=f"t3_{t}_{hf}",
                                       tag="tmp")
                    nc.vector.tensor_tensor(
                        t3[:], s[:, H + hf * HH:H + (hf + 1) * HH], s[:, hs],
                        AOP.mult)
                    nc.vector.tensor_tensor(h_new[:, hs], zh[:, hs], t3[:],
                                            AOP.add)
                    for k in range(hf * KH // 2, (hf + 1) * KH // 2):
                        tp = tr_pool.tile([128, BL], F32R, name=f"tr_{t}_{k}",
                                          tag="tr")
                        nc.tensor.transpose(tp[:],
                                            h_new[:, k * 128:(k + 1) * 128],
                                            ident[0:BL, 0:BL])
                        ht = htbpool.tile([128, BL], F32R, name=f"ht_{t}_{k}",
                                          tag="htb")
                        nc.scalar.copy(ht[:], tp[:])
                        if layer < 2 or t == seq_len - 1:
                            nc.sync.dma_start(hseqT_d[k, :, t, :], ht[:])
                        ht_new[k] = ht
                ht_prev = ht_new
                h_prev = h_new
        return h_prev

    # ---- emit the three layers ----
    # NB: for layer 0 phase A the "weight" source is wih0T with K=F=64.
    def phase_a0():
        wt = wpool.tile([128, G], F32R, name="w_l0", tag="w")
        nc.sync.dma_start(wt[:F, :], wih[0][:])
        bias_t = biaspool.tile([128, G], F32, name="bias_l0", tag="bias")
        nc.sync.dma_start(bias_t[:], bias_bc[0][:])
        with (
            tc.tile_pool(name="alhs0", bufs=4) as alhs_pool,
            tc.tile_pool(name="apsum0", bufs=2, space="PSUM") as apsum_pool,
            tc.tile_pool(name="gstage0", bufs=3) as gstage_pool,
        ):
            for j in range(nch_blocks):
                lt = alhs_pool.tile([128, 128], F32R, name=f"a0lhs_{j}",
                                    tag="alhs")
                nc.sync.dma_start(
                    lt[:F, :],
                    xT[:, 2 * j:2 * j + 2, :].rearrange("f t b -> f (t b)"))
                for n in range(NCH):
                    nsl = slice(n * 512, (n + 1) * 512)
                    ps = apsum_pool.tile([128, 512], F32, name=f"a0ps_{j}_{n}",
                                         tag="apsum")
                    nc.tensor.matmul(ps[:], lt[:F, :],
                                     wt[:F, nsl],
                                     start=True, stop=True)
                    st = gstage_pool.tile([128, 512], F32, name=f"a0st_{j}_{n}",
                                          tag="gst")
                    nc.vector.tensor_tensor(st[:], ps[:], bias_t[:, nsl], AOP.add)
                    nc.sync.dma_start(gx_d[j * 128:(j + 1) * 128, nsl], st[:])

    def phase_a_l(layer):
        wtiles = load_w_chunks(wih[layer])
        bias_t = biaspool.tile([128, G], F32, name=f"bias_l{layer}", tag="bias")
        nc.sync.dma_start(bias_t[:], bias_bc[layer][:])
        with (
            tc.tile_pool(name=f"alhs{layer}", bufs=2 * KH) as alhs_pool,
            tc.tile_pool(name=f"apsum{layer}", bufs=2, space="PSUM") as apsum_pool,
            tc.tile_pool(name=f"gstage{layer}", bufs=3) as gstage_pool,
        ):
            for j in range(nch_blocks):
                lhs = []
                for k in range(KH):
                    lt = alhs_pool.tile([128, 128], F32R,
                                        name=f"alhs{layer}_{j}_{k}", tag="alhs")
                    nc.sync.dma_start(
                        lt[:],
                        hseqT_d[k, :, 2 * j:2 * j + 2, :].rearrange(
                            "p t b -> p (t b)"))
                    lhs.append(lt)
                for n in range(NCH):
                    nsl = slice(n * 512, (n + 1) * 512)
                    ps = apsum_pool.tile([128, 512], F32,
                                         name=f"aps{layer}_{j}_{n}", tag="apsum")
                    for k in range(KH):
                        wt, _ = wtiles[k]
                        nc.tensor.matmul(ps[:], lhs[k][:],
                                         wt[:, nsl],
                                         start=(k == 0), stop=(k == KH - 1))
                    st = gstage_pool.tile([128, 512], F32,
                                          name=f"ast{layer}_{j}_{n}", tag="gst")
                    nc.vector.tensor_tensor(st[:], ps[:], bias_t[:, nsl], AOP.add)
                    nc.sync.dma_start(gx_d[j * 128:(j + 1) * 128, nsl], st[:])

    phase_a0()
    phase_r(0)
    phase_a_l(1)
    phase_r(1)
    phase_a_l(2)
    h_last = phase_r(2)

    # ---- BatchNorm stats + BN/FC folded head ----
    with (
        tc.tile_pool(name="bnps", bufs=1, space="PSUM") as bn_psum,
        tc.tile_pool(name="bnsb", bufs=1) as bn_sb,
    ):
        ones_col = constt[0:BL, 128:129].bitcast(F32)
        h_sq = bn_sb.tile([BL, H], F32, name="h_sq")
        nc.scalar.activation(h_sq[:], h_last[:], ACTF.Square)

        stats_ps = bn_psum.tile([128, 2 * KH], F32, name="stats_ps", tag="bnp")
        for k in range(KH):
            ksl = slice(k * 128, (k + 1) * 128)
            nc.tensor.matmul(stats_ps[:, k:k + 1],
                             h_last[:, ksl].bitcast(F32),
                             ones_col, start=True, stop=True)
            nc.tensor.matmul(stats_ps[:, KH + k:KH + k + 1],
                             h_sq[:, ksl],
                             ones_col, start=True, stop=True)
        stats_sb = bn_sb.tile([128, 2 * KH], F32, name="stats_sb")
        nc.scalar.copy(stats_sb[:], stats_ps[:])
        nc.sync.dma_start(bn_in.rearrange("(p f) -> p f", p=128), stats_sb[:])
        nc.gpsimd.collective_compute(
            "AllReduce", AOP.add,
            replica_groups=[list(range(N_CORES))],
            ins=[bn_in[:]], outs=[bn_out[:]])
        agg = bn_sb.tile([128, 2 * KH], F32, name="agg")
        nc.sync.dma_start(agg[:], bn_out.rearrange("(p f) -> p f", p=128))

        gpm = bn_sb.tile([128, KH], F32, name="gpm")
        nc.sync.dma_start(gpm[:], gamma_pm[:])
        bpm = bn_sb.tile([128, KH], F32, name="bpm")
        nc.sync.dma_start(bpm[:], beta_pm[:])
        wpm = bn_sb.tile([128, KH], F32, name="wpm")
        nc.sync.dma_start(wpm[:], fcw_pm[:])
        fcb_t = bn_sb.tile([1, 1], F32, name="fcb_t")
        nc.sync.dma_start(fcb_t[:], fcb_d[:])

        mu = bn_sb.tile([128, KH], F32, name="mu")
        nc.scalar.mul(mu[:], agg[:, 0:KH], 1.0 / B)
        ex2 = bn_sb.tile([128, KH], F32, name="ex2")
        nc.scalar.mul(ex2[:], agg[:, KH:2 * KH], 1.0 / B)
        musq = bn_sb.tile([128, KH], F32, name="musq")
        nc.vector.tensor_tensor(musq[:], mu[:], mu[:], AOP.mult)
        var = bn_sb.tile([128, KH], F32, name="var")
        nc.vector.tensor_tensor(var[:], ex2[:], musq[:], AOP.subtract)
        eps_t = bn_sb.tile([128, 1], F32, name="eps_t")
        nc.gpsimd.memset(eps_t[:], EPS)
        std = bn_sb.tile([128, KH], F32, name="std")
        nc.scalar.activation(std[:], var[:], ACTF.Sqrt, bias=eps_t[:])
        rstd = bn_sb.tile([128, KH], F32, name="rstd")
        nc.vector.reciprocal(rstd[:], std[:])
        scoef = bn_sb.tile([128, KH], F32, name="scoef")
        nc.vector.tensor_tensor(scoef[:], rstd[:], gpm[:], AOP.mult)
        sw = bn_sb.tile([128, KH], F32, name="sw")
        nc.vector.tensor_tensor(sw[:], scoef[:], wpm[:], AOP.mult)
        ms = bn_sb.tile([128, KH], F32, name="ms")
        nc.vector.tensor_tensor(ms[:], mu[:], scoef[:], AOP.mult)
        d = bn_sb.tile([128, KH], F32, name="d")
        nc.vector.tensor_tensor(d[:], bpm[:], ms[:], AOP.subtract)
        dw = bn_sb.tile([128, KH], F32, name="dw")
        nc.vector.tensor_tensor(dw[:], d[:], wpm[:], AOP.mult)
        dw1 = bn_sb.tile([128, 1], F32, name="dw1")
        nc.vector.reduce_sum(dw1[:], dw[:], mybir.AxisListType.X)
        ones128 = constt[:, 128:129].bitcast(F32)
        c_ps = bn_psum.tile([1, 1], F32, name="c_ps", tag="bnc")
        nc.tensor.matmul(c_ps[:], dw1[:],
                         ones128, start=True, stop=True)
        c_sb = bn_sb.tile([1, 1], F32, name="c_sb")
        nc.vector.tensor_tensor(c_sb[:], c_ps[:], fcb_t[:], AOP.add)

        # y = h63 @ sw + C   via hT63 chunks (stored to hseqT_d at t=T-1)
        y_ps = bn_psum.tile([BL, 1], F32, name="y_ps", tag="bny")
        ht63 = []
        for k in range(KH):
            htk = bn_sb.tile([128, BL], F32, name=f"ht63_{k}")
            nc.sync.dma_start(htk[:], hseqT_d[k, :, seq_len - 1, :].bitcast(F32))
            ht63.append(htk)
        for k in range(KH):
            nc.tensor.matmul(y_ps[:], ht63[k][:],
                             sw[:, k:k + 1],
                             start=(k == 0), stop=False)
        onesb = constt[0:1, 128:128 + BL].bitcast(F32)
        nc.tensor.matmul(y_ps[:], onesb,
                         c_sb[:], start=False, stop=True)
        res = bn_sb.tile([BL, 1], F32, name="res")
        nc.scalar.activation(res[:], y_ps[:], ACTF.Sigmoid)
        nc.sync.dma_start(out_d.rearrange("(p f) -> p f", f=1), res[:])

    miscpool.release()
    biaspool.release()
    htbpool.release()
    hpool.release()
    wpool.release()
    const_pool.release()


_PROGRAM_CACHE = {}


def build_program(seq_len=T):
    if seq_len in _PROGRAM_CACHE:
        return _PROGRAM_CACHE[seq_len]
    nc = bacc.Bacc("TRN2", target_bir_lowering=False, debug=False,
                   num_devices=N_CORES)
    with nc.allow_low_precision(reason="fp32r state/operands are intentional"):
        with tile.TileContext(nc) as tc:
            _emit(nc, tc, seq_len)
    nc.compile()
    _PROGRAM_CACHE[seq_len] = nc
    return nc


def make_in_maps(inputs, seq_len=T):
    f32 = np.float32

    def prep_shared():
        m = {}
        m["wih0T"] = np.ascontiguousarray(inputs["Wih0"].T, dtype=f32)
        m["wih1T"] = np.ascontiguousarray(inputs["Wih1"].T, dtype=f32)
        m["wih2T"] = np.ascontiguousarray(inputs["Wih2"].T, dtype=f32)
        for i in range(3):
            m[f"whh{i}T"] = np.ascontiguousarray(
                inputs[f"Whh{i}"].T, dtype=f32)
            bih = np.asarray(inputs[f"bih{i}"], dtype=f32)
            bhh = np.asarray(inputs[f"bhh{i}"], dtype=f32)
            bias = bih.copy()
            bias[:2 * H] += bhh[:2 * H]
            m[f"bias{i}"] = np.ascontiguousarray(
                np.broadcast_to(bias, (128, G)), dtype=f32)
            misc = np.zeros((1, 1088), dtype=f32)
            misc[0, :H] = bhh[2 * H:]
            misc[0, H:H + 64] = 1.0
            m[f"misc{i}"] = misc
        for name, key in (("gamma_pm", "gamma"), ("beta_pm", "beta")):
            v = np.asarray(inputs[key], dtype=f32)
            m[name] = np.ascontiguousarray(v.reshape(KH, 128).T)
        fcw = np.asarray(inputs["fcW"], dtype=f32).reshape(H)
        m["fcw_pm"] = np.ascontiguousarray(fcw.reshape(KH, 128).T)
        m["fcb"] = np.asarray(inputs["fcb"], dtype=f32).reshape(1, 1)
        cd = np.zeros((128, 256), dtype=f32)
        cd[:, :128] = np.eye(128, dtype=f32)
        cd[:, 128:192] = 1.0
        m["const_d"] = cd
        return m

    shared = prep_shared()
    x = np.asarray(inputs["x"], dtype=f32)
    in_maps = []
    for c in range(N_CORES):
        xs = x[c * BL:(c + 1) * BL, :seq_len, :]          # [BL, T, F]
        xT_c = np.ascontiguousarray(xs.transpose(2, 1, 0))  # [F, T, BL]
        m = dict(shared)
        m["xT"] = xT_c
        in_maps.append(m)
    return in_maps


def kernel(**inputs):
    nc = build_program(T)
    in_maps = make_in_maps(inputs, T)
    res = run_bass_kernel_spmd(nc, in_maps, list(range(N_CORES)))
    out = np.concatenate([res.results[c]["out"] for c in range(N_CORES)])
    return out.astype(np.float32)
